# revision 11
# baseline (speedup 1.0000x reference)
"""CRF loss (forward-algorithm partition function) on 8 Trainium2 cores.

Strategy
--------
Batch (B=64) is sharded 8 ways -> 8 sequences per core.  The lax.scan
over L=512 steps is computed in *linear* space: with

    E_l = exp(scores_l - C),   C = log(T) + 0.5

the log-space recurrence  p_{l}[t'] = logsumexp_t(scores_l[t,t'] + p_{l-1}[t])
becomes  w_l = E_l^T w_{l-1},  with  p_l = log(w_l) + s0 + l*C  recovered at
the end (drift of log|w| stays within +-1 for N(0,1) scores, so fp32 is safe
-- validated to ~2.5e-6 absolute partition error).

Per core the 511-step chain is run as tiny TensorE matvecs: the exp'd score
tile for two batch rows is packed [128=(2b x 64t), 64=t'] and used as the
*stationary* operand (lhsT); the running vector w is the N=1 moving operand;
the output column lands in PSUM and one [128,8] DVE copy per step moves all
8 batch rows' new vectors back to SBUF.  exp() is done by ScalarE on big
[128, 32*64] tiles, off the critical path.

The tiny remainder (gold-path gather, softmax weight, final log/sum) is done
on the host -- it touches 0.02% of the data.
"""

import os
import threading
import numpy as np

L, B, T = 512, 64, 64
NCORES = 8
B_LOC = B // NCORES            # 8 sequences per core
NPAIR = B_LOC // 2             # 4 partition-pairs per core
NSTEP = L - 1                  # 511 chain steps (l = 1..511)
KB = 32                        # chain steps exp'd/DMA'd per block
C_SHIFT = float(np.log(T) + 0.5)
START_TAG = 0
END_TAG = 1

_nc_cache = [None]
_nc_lock = threading.Lock()
LAST_RESULTS = [None]          # test.py reads exec_time_ns from here


def _build_nc():
    import concourse.bacc as bacc
    import concourse.mybir as mybir
    import concourse.tile as tile

    dt = mybir.dt
    nc = bacc.Bacc("TRN2", target_bir_lowering=False, debug=False)

    scores_d = nc.declare_dram_parameter(
        "scores_loc", [L, B_LOC, T, T], dt.float32, isOutput=False
    )
    rhs_init_d = nc.declare_dram_parameter(
        "rhs_init", [128, 16], dt.float32, isOutput=False
    )
    out_d = nc.declare_dram_parameter("w_out", [128, 16], dt.float32, isOutput=True)

    blocks = []
    l0 = 1
    while l0 < L:
        nst = min(KB, L - l0)
        blocks.append((l0, nst))
        l0 += nst

    with tile.TileContext(nc) as tc:
        with (
            tc.tile_pool(name="exp", bufs=2) as exp_pool,
            tc.tile_pool(name="state", bufs=1) as state_pool,
            tc.tile_pool(name="psum", bufs=1, space="PSUM") as psum_pool,
        ):
            rhs = state_pool.tile([128, 16], dt.float32)
            zeros = state_pool.tile([128, 16], dt.float32)
            psum = psum_pool.tile([128, 16], dt.float32)

            nc.sync.dma_start(rhs[:], rhs_init_d[:])
            nc.vector.memset(zeros[:], 0.0)
            # Pre-zero PSUM once: matvec outputs only ever write the
            # [0:64, even-col] / [64:128, odd-col] windows, so the
            # complementary windows stay exactly 0 forever and the per-step
            # [128,8] copy propagates those zeros into the rhs zero slots.
            nc.vector.tensor_copy(psum[:], zeros[:])

            step = 0
            for (l0, nst) in blocks:
                tiles = []
                for q in range(NPAIR):
                    t = exp_pool.tile([128, nst * T], dt.float32, tag=f"pair{q}")
                    src = scores_d[l0 : l0 + nst, 2 * q : 2 * q + 2, :, :].rearrange(
                        "j b t u -> (b t) j u"
                    )
                    dst = t[:].rearrange("p (j u) -> p j u", u=T)
                    nc.sync.dma_start(dst, src)
                    # plain exp (no bias AP -> one less wait on the ACT
                    # instruction); the e^{-C} normalization is folded into
                    # the per-step DVE copy-back instead
                    nc.scalar.activation(t[:], t[:], mybir.ActivationFunctionType.Exp)
                    tiles.append(t)
                for j in range(nst):
                    ph = step % 2
                    ph2 = (step + 1) % 2
                    for q in range(NPAIR):
                        lhsT = tiles[q][:, j * T : (j + 1) * T]
                        c_r = ph * 8 + 2 * q
                        c_w = ph2 * 8 + 2 * q
                        nc.tensor.matmul(
                            psum[0:64, c_w : c_w + 1],
                            lhsT,
                            rhs[:, c_r : c_r + 1],
                            start=True,
                            stop=True,
                        )
                        nc.tensor.matmul(
                            psum[64:128, c_w + 1 : c_w + 2],
                            lhsT,
                            rhs[:, c_r + 1 : c_r + 2],
                            start=True,
                            stop=True,
                        )
                    nc.vector.tensor_scalar_mul(
                        rhs[:, ph2 * 8 : ph2 * 8 + 8],
                        psum[:, ph2 * 8 : ph2 * 8 + 8],
                        float(np.exp(-C_SHIFT)),
                    )
                    step += 1

            nc.sync.dma_start(out_d[:], rhs[:])
    nc.compile()
    return nc


def _get_nc():
    with _nc_lock:
        if _nc_cache[0] is None:
            _nc_cache[0] = _build_nc()
        return _nc_cache[0]


def _ensure_axon_hooks():
    """Provide antenv.axon_hooks (missing in this image) so that
    run_bass_kernel_spmd(trace=True) can register the NTFF profile hook."""
    import sys
    import types

    try:
        import antenv.axon_hooks  # noqa: F401
        return
    except ImportError:
        pass
    import antenv

    mod = types.ModuleType("antenv.axon_hooks")
    _hook = [None]
    mod.set_axon_ntff_profile_hook = lambda h: _hook.__setitem__(0, h)
    mod.get_axon_ntff_profile_hook = lambda: _hook[0]
    sys.modules["antenv.axon_hooks"] = mod
    antenv.axon_hooks = mod
    try:
        from trn_agent_boot.trn_boot import _ntff_profile_via_ctypes

        h = _ntff_profile_via_ctypes("/opt/axon/libaxon_pjrt.so")
        if h is not None:
            mod.set_axon_ntff_profile_hook(h)
    except Exception:
        pass


def kernel(scores, target, mask, antor_score, aid, **_unused):
    from concourse.bass_utils import run_bass_kernel_spmd

    scores = np.asarray(scores, dtype=np.float32)
    target = np.asarray(target)
    mask = np.asarray(mask)
    antor_score = np.asarray(antor_score, dtype=np.float32)
    aid = int(np.asarray(aid))
    assert scores.shape == (L, B, T, T), scores.shape

    mask_all = bool(mask.all())

    # ---- host prep: shard batch, build initial vectors ----
    p0 = scores[0, :, START_TAG, :].astype(np.float64)          # (B, T)
    s0 = p0.max(axis=1)                                          # (B,)
    w0 = np.exp(p0 - s0[:, None]).astype(np.float32)             # (B, T)

    def make_shard(c):
        sh = np.ascontiguousarray(scores[:, c * B_LOC : (c + 1) * B_LOC])
        if not mask_all:
            # a masked step must leave the partition unchanged:
            # E = e^{-C} * I  <=>  scores_eff = 0 on diag, -inf off-diag
            mloc = mask[:, c * B_LOC : (c + 1) * B_LOC]
            eye = np.full((T, T), -1e30, dtype=np.float32)
            np.fill_diagonal(eye, 0.0)
            ls, lb = np.nonzero(~mloc)
            sh[ls, lb] = eye
        return sh

    shards = [None] * NCORES
    threads = [
        threading.Thread(target=lambda c=c: shards.__setitem__(c, make_shard(c)))
        for c in range(NCORES)
    ]
    for t in threads:
        t.start()
    for t in threads:
        t.join()

    in_maps = []
    for c in range(NCORES):
        rhs_init = np.zeros((128, 16), dtype=np.float32)
        for b in range(B_LOC):
            q, half = b // 2, b % 2
            col = 2 * q + half
            rhs_init[half * 64 : half * 64 + 64, col] = w0[c * B_LOC + b]
        in_maps.append({"scores_loc": shards[c], "rhs_init": rhs_init})

    nc = _get_nc()
    do_trace = bool(int(os.environ.get("KERNEL_TRACE", "0")))
    if do_trace:
        _ensure_axon_hooks()
    try:
        res = run_bass_kernel_spmd(nc, in_maps, list(range(NCORES)), trace=do_trace)
    except Exception:
        if not do_trace:
            raise
        res = run_bass_kernel_spmd(nc, in_maps, list(range(NCORES)), trace=False)
    LAST_RESULTS[0] = res

    # ---- host finish ----
    parity = NSTEP % 2  # phase the final copy wrote into
    Z = 0.0
    for c in range(NCORES):
        out = res.results[c]["w_out"]
        for b in range(B_LOC):
            q, half = b // 2, b % 2
            col = parity * 8 + 2 * q + half
            w_end = float(out[half * 64 + END_TAG, col])
            Z += np.log(w_end) + s0[c * B_LOC + b] + NSTEP * C_SHIFT

    maskf = mask.astype(np.float64)
    tg = np.take_along_axis(
        scores.reshape(L, B, T * T), np.asarray(target, np.int64)[:, :, None], axis=2
    )[..., 0]
    tg_energy = float((tg * maskf).sum())

    a = antor_score.astype(np.float64)
    wsm = np.exp(a - a.max())
    wsm /= wsm.sum()
    loss = (Z - tg_energy) * wsm[aid] / B
    return np.float32(loss)


# revision 14
# speedup vs baseline: 1.8474x; 1.8474x over previous
"""CRF loss (forward-algorithm partition function) on 8 Trainium2 cores.

Strategy
--------
Batch (B=64) is sharded 8 ways -> 8 sequences per core.  The lax.scan
over L=512 steps is computed in *linear* space: with

    E_l = exp(scores_l - C),   C = log(T) + 0.5

the log-space recurrence  p_{l}[t'] = logsumexp_t(scores_l[t,t'] + p_{l-1}[t])
becomes  w_l = E_l^T w_{l-1},  with  p_l = log(w_l) + s0 + l*C  recovered at
the end (drift of log|w| stays within +-1 for N(0,1) scores, so fp32 is safe
-- validated to ~2.5e-6 absolute partition error).

Per core the 511-step chain is run as tiny TensorE matvecs: the exp'd score
tile for two batch rows is packed [128=(2b x 64t), 64=t'] and used as the
*stationary* operand (lhsT); the running vector w is the N=1 moving operand;
the output column lands in PSUM and one [128,8] DVE copy per step moves all
8 batch rows' new vectors back to SBUF.  exp() is done by ScalarE on big
[128, 32*64] tiles, off the critical path.

The tiny remainder (gold-path gather, softmax weight, final log/sum) is done
on the host -- it touches 0.02% of the data.
"""

import os
import threading
import numpy as np

L, B, T = 512, 64, 64
NCORES = 8
B_LOC = B // NCORES            # 8 sequences per core
NPAIR = B_LOC // 2             # 4 partition-pairs per core
NSTEP = L - 1                  # 511 chain steps (l = 1..511)
KB = 32                        # chain steps exp'd/DMA'd per block
C_SHIFT = float(np.log(T) + 0.5)
START_TAG = 0
END_TAG = 1

_nc_cache = [None]
_nc_lock = threading.Lock()
LAST_RESULTS = [None]          # test.py reads exec_time_ns from here


def _build_nc():
    import concourse.bacc as bacc
    import concourse.mybir as mybir
    import concourse.tile as tile

    dt = mybir.dt
    nc = bacc.Bacc("TRN2", target_bir_lowering=False, debug=False)

    scores_d = nc.declare_dram_parameter(
        "scores_loc", [L, B_LOC, T, T], dt.float32, isOutput=False
    )
    rhs_init_d = nc.declare_dram_parameter(
        "rhs_init", [128, 16], dt.float32, isOutput=False
    )
    out_d = nc.declare_dram_parameter("w_out", [128, 8], dt.float32, isOutput=True)

    blocks = []
    l0 = 1
    while l0 < L:
        nst = min(KB, L - l0)
        blocks.append((l0, nst))
        l0 += nst

    with tile.TileContext(nc) as tc:
        with (
            tc.tile_pool(name="raw", bufs=2) as raw_pool,
            tc.tile_pool(name="exp", bufs=2) as exp_pool,
            tc.tile_pool(name="state", bufs=1) as state_pool,
            tc.tile_pool(name="psum", bufs=1, space="PSUM") as psum_pool,
        ):
            rhs = state_pool.tile([128, 16], dt.bfloat16)
            rhs_stage = state_pool.tile([128, 16], dt.float32)
            zeros = state_pool.tile([128, 16], dt.float32)
            out_stage = state_pool.tile([128, 8], dt.float32)
            psum = psum_pool.tile([128, 16], dt.float32)

            nc.sync.dma_start(rhs_stage[:], rhs_init_d[:])
            nc.vector.tensor_copy(rhs[:], rhs_stage[:])  # fp32 -> bf16
            nc.vector.memset(zeros[:], 0.0)
            # Pre-zero PSUM once: matvec outputs only ever write the
            # [0:64, even-col] / [64:128, odd-col] windows, so the
            # complementary windows stay exactly 0 forever and the per-step
            # [128,8] copy propagates those zeros into the rhs zero slots.
            nc.vector.tensor_copy(psum[:], zeros[:])

            step = 0
            for (l0, nst) in blocks:
                tiles = []
                for q in range(NPAIR):
                    t_raw = raw_pool.tile([128, nst * T], dt.float32, tag=f"raw{q}")
                    t = exp_pool.tile([128, nst * T], dt.bfloat16, tag=f"pair{q}")
                    src = scores_d[l0 : l0 + nst, 2 * q : 2 * q + 2, :, :].rearrange(
                        "j b t u -> (b t) j u"
                    )
                    dst = t_raw[:].rearrange("p (j u) -> p j u", u=T)
                    nc.sync.dma_start(dst, src)
                    # bf16 exp output: single-pass LDWEIGHTS/MATMUL on the PE
                    # (fp32 would run in double-pass LOW_HIGH mode).  The
                    # e^{-C} normalization is folded into the per-step DVE
                    # copy-back instead of an ACT bias.
                    nc.scalar.activation(
                        t[:], t_raw[:], mybir.ActivationFunctionType.Exp
                    )
                    tiles.append(t)
                for j in range(nst):
                    ph = step % 2
                    ph2 = (step + 1) % 2
                    for q in range(NPAIR):
                        lhsT = tiles[q][:, j * T : (j + 1) * T]
                        c_r = ph * 8 + 2 * q
                        c_w = ph2 * 8 + 2 * q
                        nc.tensor.matmul(
                            psum[0:64, c_w : c_w + 1],
                            lhsT,
                            rhs[:, c_r : c_r + 1],
                            start=True,
                            stop=True,
                        )
                        nc.tensor.matmul(
                            psum[64:128, c_w + 1 : c_w + 2],
                            lhsT,
                            rhs[:, c_r + 1 : c_r + 2],
                            start=True,
                            stop=True,
                        )
                    nc.vector.tensor_scalar_mul(
                        rhs[:, ph2 * 8 : ph2 * 8 + 8],
                        psum[:, ph2 * 8 : ph2 * 8 + 8],
                        float(np.exp(-C_SHIFT)),
                    )
                    step += 1

            # export the final *unscaled* fp32 accumulator (one e^{-C} is
            # still owed; the host applies it in log space)
            parity = NSTEP % 2
            nc.vector.tensor_copy(out_stage[:], psum[:, parity * 8 : parity * 8 + 8])
            nc.sync.dma_start(out_d[:], out_stage[:])
    nc.compile()
    return nc


def _get_nc():
    with _nc_lock:
        if _nc_cache[0] is None:
            _nc_cache[0] = _build_nc()
        return _nc_cache[0]


def _ensure_axon_hooks():
    """Provide antenv.axon_hooks (missing in this image) so that
    run_bass_kernel_spmd(trace=True) can register the NTFF profile hook."""
    import sys
    import types

    try:
        import antenv.axon_hooks  # noqa: F401
        return
    except ImportError:
        pass
    import antenv

    mod = types.ModuleType("antenv.axon_hooks")
    _hook = [None]
    mod.set_axon_ntff_profile_hook = lambda h: _hook.__setitem__(0, h)
    mod.get_axon_ntff_profile_hook = lambda: _hook[0]
    sys.modules["antenv.axon_hooks"] = mod
    antenv.axon_hooks = mod
    try:
        from trn_agent_boot.trn_boot import _ntff_profile_via_ctypes

        h = _ntff_profile_via_ctypes("/opt/axon/libaxon_pjrt.so")
        if h is not None:
            mod.set_axon_ntff_profile_hook(h)
    except Exception:
        pass


def kernel(scores, target, mask, antor_score, aid, **_unused):
    from concourse.bass_utils import run_bass_kernel_spmd

    scores = np.asarray(scores, dtype=np.float32)
    target = np.asarray(target)
    mask = np.asarray(mask)
    antor_score = np.asarray(antor_score, dtype=np.float32)
    aid = int(np.asarray(aid))
    assert scores.shape == (L, B, T, T), scores.shape

    mask_all = bool(mask.all())

    # ---- host prep: shard batch, build initial vectors ----
    p0 = scores[0, :, START_TAG, :].astype(np.float64)          # (B, T)
    s0 = p0.max(axis=1)                                          # (B,)
    w0 = np.exp(p0 - s0[:, None]).astype(np.float32)             # (B, T)

    def make_shard(c):
        sh = np.ascontiguousarray(scores[:, c * B_LOC : (c + 1) * B_LOC])
        if not mask_all:
            # a masked step must leave the partition unchanged:
            # E = e^{-C} * I  <=>  scores_eff = 0 on diag, -inf off-diag
            mloc = mask[:, c * B_LOC : (c + 1) * B_LOC]
            eye = np.full((T, T), -1e30, dtype=np.float32)
            np.fill_diagonal(eye, 0.0)
            ls, lb = np.nonzero(~mloc)
            sh[ls, lb] = eye
        return sh

    shards = [None] * NCORES
    threads = [
        threading.Thread(target=lambda c=c: shards.__setitem__(c, make_shard(c)))
        for c in range(NCORES)
    ]
    for t in threads:
        t.start()
    for t in threads:
        t.join()

    in_maps = []
    for c in range(NCORES):
        rhs_init = np.zeros((128, 16), dtype=np.float32)
        for b in range(B_LOC):
            q, half = b // 2, b % 2
            col = 2 * q + half
            rhs_init[half * 64 : half * 64 + 64, col] = w0[c * B_LOC + b]
        in_maps.append({"scores_loc": shards[c], "rhs_init": rhs_init})

    nc = _get_nc()
    do_trace = bool(int(os.environ.get("KERNEL_TRACE", "0")))
    if do_trace:
        _ensure_axon_hooks()
    try:
        res = run_bass_kernel_spmd(nc, in_maps, list(range(NCORES)), trace=do_trace)
    except Exception:
        if not do_trace:
            raise
        res = run_bass_kernel_spmd(nc, in_maps, list(range(NCORES)), trace=False)
    LAST_RESULTS[0] = res

    # ---- host finish ----
    # w_out holds the final step's *unscaled* accumulator: one e^{-C} is
    # still owed, i.e. partition = log(acc) - C + s0 + NSTEP*C
    Z = 0.0
    for c in range(NCORES):
        out = res.results[c]["w_out"]
        for b in range(B_LOC):
            q, half = b // 2, b % 2
            acc_end = float(out[half * 64 + END_TAG, 2 * q + half])
            Z += np.log(acc_end) + s0[c * B_LOC + b] + (NSTEP - 1) * C_SHIFT

    maskf = mask.astype(np.float64)
    tg = np.take_along_axis(
        scores.reshape(L, B, T * T), np.asarray(target, np.int64)[:, :, None], axis=2
    )[..., 0]
    tg_energy = float((tg * maskf).sum())

    a = antor_score.astype(np.float64)
    wsm = np.exp(a - a.max())
    wsm /= wsm.sum()
    loss = (Z - tg_energy) * wsm[aid] / B
    return np.float32(loss)
